# revision 1
# baseline (speedup 1.0000x reference)
"""AttentionBlockWithSkipConnection Trainium2 kernel.

Full inputs -> full output. Data-parallel over batch B=8 across 8 cores.
Each core computes one batch: GroupNorm -> qkv 1x1conv -> full 4096x4096
attention -> proj 1x1conv -> skip add.

Layout strategy: channel-major ("transposed") throughout the middle of the
pipeline so every matmul contracts over the partition dim and the 4096x4096
attention matrix is never transposed or spilled:
  x^T [C, N]           (C=256 as 2 partition-chunks of 128; 64 PE transposes)
  GroupNorm folded into the qkv weights: h = a*x + b (per channel) =>
      qkv^T = (w*a)^T @ x^T + (w^T b + b_qkv)
  logits^T[k,q] = (K^T).T @ Q^T        (both operands channel-major)
  expT = exp(logits^T / 16)            (softmax denominator = partition sums,
                                        accumulated on DVE, finished by a
                                        ones-row matmul)
  o_un^T = V.T @ expT                  (V token-major via 64 PE transposes,
                                        flash-style PSUM accumulation)
  proj_un^T = w_proj.T @ o_un^T
  proj^T = proj_un^T * (1/colsum broadcast via all-ones matmul) + b_proj
  out = transpose(proj^T) + x

All heavy matmuls run in float32r (~4x the fp32 rate for free dims >= 256,
~1.6e-4 relative error); producers round-on-write as walrus requires.
"""

import numpy as np

import concourse.bacc as bacc
import concourse.mybir as mybir
import concourse.tile as tile

N_CORES = 8
B, H, W, C = 8, 64, 64, 256
N = H * W  # 4096 tokens
G = 32  # groups
GS = C // G  # 8 channels per group
EPS = 1e-5
CC = C // 128  # 2 channel chunks
QT = 512  # q tile (free dim of logits/attnv matmuls)
NQ = N // QT  # 8
NK = N // 128  # 32 k tiles
F32 = mybir.dt.float32

USE_F32R = True


def _mm(ap):
    """Matmul-input view: fp32 data consumed as float32r."""
    if USE_F32R:
        return ap.bitcast(mybir.dt.float32r)
    return ap


def _rw(ap):
    """Round-on-write view: engine writes through this AP round to fp32r,
    which the walrus verifier requires for fp32r matmul inputs."""
    if USE_F32R:
        return ap.bitcast(mybir.dt.float32r)
    return ap


def _build(repeat=1):
    nc = bacc.Bacc(
        "TRN2",
        target_bir_lowering=False,
        debug=False,
        enable_asserts=True,
        num_devices=N_CORES,
    )
    x_d = nc.dram_tensor("x", [N, C], F32, kind="ExternalInput")
    gns_d = nc.dram_tensor("gn_scale", [C], F32, kind="ExternalInput")
    gnb_d = nc.dram_tensor("gn_bias", [C], F32, kind="ExternalInput")
    wq_d = nc.dram_tensor("w_qkv", [C, 3 * C], F32, kind="ExternalInput")
    bq_d = nc.dram_tensor("b_qkv", [3 * C], F32, kind="ExternalInput")
    wp_d = nc.dram_tensor("w_proj", [C, C], F32, kind="ExternalInput")
    bp_d = nc.dram_tensor("b_proj", [C], F32, kind="ExternalInput")
    out_d = nc.dram_tensor("out", [N, C], F32, kind="ExternalOutput")

    # group-aggregation masks: gA averages 8 consecutive partitions into one
    # group row; gB broadcasts group rows back to their 128 channels.
    gA_np = np.zeros((128, 16), np.float32)
    gB_np = np.zeros((16, 128), np.float32)
    for p in range(128):
        gA_np[p, p // GS] = 1.0 / GS
        gB_np[p // GS, p] = 1.0
    gA_d = nc.inline_tensor(gA_np, "gA")
    gB_d = nc.inline_tensor(gB_np, "gB")
    ident_d = nc.inline_tensor(np.eye(128, dtype=np.float32), "ident")

    with tile.TileContext(nc) as tc:
        for _ in range(repeat):
            _body(tc, x_d, gns_d, gnb_d, wq_d, bq_d, wp_d, bp_d, out_d,
                  gA_d, gB_d, ident_d)
    nc.compile()
    return nc


def _body(tc, x_d, gns_d, gnb_d, wq_d, bq_d, wp_d, bp_d, out_d,
          gA_d, gB_d, ident_d):
    nc = tc.nc
    x_tok = x_d.ap().rearrange("(p nt) c -> p nt c", p=128)  # [128, 32, 256]
    out_tok = out_d.ap().rearrange("(p nt) c -> p nt c", p=128)

    with (
        tc.tile_pool(name="consts", bufs=1) as consts,
        tc.tile_pool(name="psum_tr", bufs=2, space="PSUM") as psum_tr,
        tc.tile_pool(name="psum_mm", bufs=4, space="PSUM") as psum_mm,
        tc.tile_pool(name="psum_acc", bufs=1, space="PSUM") as psum_acc,
        tc.tile_pool(name="qkvT", bufs=1) as qkvT_pool,
    ):
        # ---- input DMAs: x first (PE transposes gate on it), identity on
        # the same fast HWDGE queue, weights on the cheap GPSIMD queue ----
        ident = consts.tile([128, 128], F32)
        nc.sync.dma_start(out=ident, in_=ident_d.ap())
        qkvT = qkvT_pool.tile([128, 6, N], F32)  # 96KB/partition

        with (
            tc.tile_pool(name="xcm", bufs=1) as xcm_pool,
            tc.tile_pool(name="xtm", bufs=1) as xtm_pool,
            tc.tile_pool(name="gn_stats", bufs=2) as gn_stats,
        ):
            x_cm = xcm_pool.tile([128, CC, N], F32)  # 32KB/partition
            x_tm = xtm_pool.tile([128, 32, C], F32)  # 32KB/partition
            dma_engs = [nc.sync, nc.scalar]
            for dchunk in range(16):
                dma_engs[dchunk % 2].dma_start(
                    out=x_tm[:, dchunk * 2 : (dchunk + 1) * 2, :],
                    in_=x_tok[:, dchunk * 2 : (dchunk + 1) * 2, :],
                )

            # ---- weights / small constants (HWDGE, behind the x chunks;
            # the GPSIMD SWDGE queue's software descriptor-gen proved slow
            # enough on hardware to sit on the critical path) ----
            gA = consts.tile([128, 16], F32)
            nc.sync.dma_start(out=gA, in_=gA_d.ap())
            gB = consts.tile([16, 128], F32)
            nc.scalar.dma_start(out=gB, in_=gB_d.ap())
            wq_stage = consts.tile([128, CC, 3 * C], F32)
            nc.sync.dma_start(
                out=wq_stage, in_=wq_d.ap().rearrange("(cc p) d -> p cc d", p=128)
            )
            wp_stage = consts.tile([128, CC, C], F32)
            nc.scalar.dma_start(
                out=wp_stage, in_=wp_d.ap().rearrange("(cc p) d -> p cc d", p=128)
            )
            wp = consts.tile([128, CC, C], F32)
            nc.vector.tensor_copy(out=_rw(wp), in_=wp_stage)
            bq = consts.tile([128, 6], F32)
            nc.sync.dma_start(
                out=bq, in_=bq_d.ap().rearrange("(m p) -> p m", p=128)
            )
            bp_col = consts.tile([128, CC], F32)
            nc.scalar.dma_start(
                out=bp_col, in_=bp_d.ap().rearrange("(dc p) -> p dc", p=128)
            )
            gns = consts.tile([128, CC], F32)
            nc.sync.dma_start(
                out=gns, in_=gns_d.ap().rearrange("(cc p) -> p cc", p=128)
            )
            gnb = consts.tile([128, CC], F32)
            nc.scalar.dma_start(
                out=gnb, in_=gnb_d.ap().rearrange("(cc p) -> p cc", p=128)
            )
            ones_raw = consts.tile([128, 128], F32)
            nc.vector.memset(ones_raw, 1.0)
            ones_mat = consts.tile([128, 128], F32)
            nc.vector.tensor_copy(out=_rw(ones_mat), in_=ones_raw)
            eps_col = consts.tile([128, 1], F32)
            nc.vector.memset(eps_col, EPS)

            # ---- phase A: transpose x to channel-major; bn_stats interleaved
            # so the statistics finish right after the last transpose ----
            stats = gn_stats.tile([128, CC, 8, 6], F32)
            for s in range(8):
                for nt in range(4 * s, 4 * s + 4):
                    for cc in range(CC):
                        ps = psum_tr.tile([128, 128], F32, tag="tr")
                        nc.tensor.transpose(
                            ps, x_tm[:, nt, cc * 128 : (cc + 1) * 128], ident
                        )
                        # alternate PSUM->SBUF copies across DVE and ACT so
                        # neither engine serializes the prologue
                        ceng = nc.vector if (nt + cc) % 2 == 0 else nc.scalar
                        if ceng is nc.vector:
                            ceng.tensor_copy(
                                out=_rw(x_cm[:, cc, nt * 128 : (nt + 1) * 128]),
                                in_=ps,
                            )
                        else:
                            nc.scalar.copy(
                                out=_rw(x_cm[:, cc, nt * 128 : (nt + 1) * 128]),
                                in_=ps,
                            )
                for cc in range(CC):
                    nc.vector.bn_stats(
                        out=stats[:, cc, s, :],
                        in_=x_cm[:, cc, s * 512 : (s + 1) * 512],
                    )

            # ---- groupnorm stats -> per-channel affine (a, b) ----
            ab = gn_stats.tile([128, CC, 2], F32)  # (a, b) per channel
            for cc in range(CC):
                mv = gn_stats.tile([128, 2], F32, tag="mv")
                nc.vector.bn_aggr(out=mv, in_=stats[:, cc, :, :])
                # mv2 = (mean, E[x^2])
                mv2 = gn_stats.tile([128, 2], F32, tag="mv2")
                nc.vector.tensor_copy(out=mv2[:, 0:1], in_=mv[:, 0:1])
                nc.vector.tensor_mul(out=mv2[:, 1:2], in0=mv[:, 0:1], in1=mv[:, 0:1])
                nc.vector.tensor_add(out=mv2[:, 1:2], in0=mv2[:, 1:2], in1=mv[:, 1:2])
                # aggregate to 16 group rows, then broadcast back to channels
                gp = psum_tr.tile([16, 2], F32, tag="tr", name="gp")
                nc.tensor.matmul(gp, lhsT=gA, rhs=mv2, start=True, stop=True)
                gp_sb = gn_stats.tile([16, 2], F32, tag="gp_sb")
                nc.vector.tensor_copy(out=gp_sb, in_=gp)
                chs = psum_tr.tile([128, 2], F32, tag="tr", name="chs")
                nc.tensor.matmul(chs, lhsT=gB, rhs=gp_sb, start=True, stop=True)
                chs_sb = gn_stats.tile([128, 2], F32, tag="chs_sb")
                nc.vector.tensor_copy(out=chs_sb, in_=chs)
                # var = E[x^2] - mean^2 ; rstd = 1/sqrt(var+eps)
                var = gn_stats.tile([128, 1], F32, tag="var")
                msq = gn_stats.tile([128, 1], F32, tag="msq")
                nc.vector.tensor_mul(out=msq, in0=chs_sb[:, 0:1], in1=chs_sb[:, 0:1])
                nc.vector.tensor_sub(out=var, in0=chs_sb[:, 1:2], in1=msq)
                nc.scalar.activation(
                    out=var,
                    in_=var,
                    func=mybir.ActivationFunctionType.Sqrt,
                    bias=eps_col,
                )
                rstd = gn_stats.tile([128, 1], F32, tag="rstd")
                nc.vector.reciprocal(out=rstd, in_=var)
                # a = rstd*gn_scale ; b = gn_bias - mean*a
                nc.vector.tensor_mul(
                    out=ab[:, cc, 0:1], in0=rstd, in1=gns[:, cc : cc + 1]
                )
                nc.vector.tensor_mul(out=msq, in0=chs_sb[:, 0:1], in1=ab[:, cc, 0:1])
                nc.vector.tensor_sub(
                    out=ab[:, cc, 1:2], in0=gnb[:, cc : cc + 1], in1=msq
                )

            # ---- fold the affine into the qkv weights:
            # qkv^T = (w*a)^T x^T + (w^T b + b_qkv) ----
            wq = consts.tile([128, CC, 3 * C], F32)
            for m in range(6):
                for cc in range(CC):
                    nc.scalar.mul(
                        out=_rw(wq[:, cc, m * 128 : (m + 1) * 128]),
                        in_=wq_stage[:, cc, m * 128 : (m + 1) * 128],
                        mul=ab[:, cc, 0:1],
                    )
            bias2 = gn_stats.tile([128, 6], F32)
            for m in range(6):
                psb = psum_tr.tile([128, 1], F32, tag="tr", name="psb")
                for cc in range(CC):
                    nc.tensor.matmul(
                        psb,
                        lhsT=wq_stage[:, cc, m * 128 : (m + 1) * 128],
                        rhs=ab[:, cc, 1:2],
                        start=(cc == 0),
                        stop=(cc == CC - 1),
                    )
                nc.vector.tensor_add(
                    out=bias2[:, m : m + 1], in0=psb, in1=bq[:, m : m + 1]
                )

            # ---- phase B: qkv^T = wq.T @ x^T (+ bias2) ----
            for m in range(6):
                for qt in range(NQ):
                    ps = psum_mm.tile([128, QT], F32, tag="mm")
                    for cc in range(CC):
                        nc.tensor.matmul(
                            ps,
                            lhsT=_mm(wq[:, cc, m * 128 : (m + 1) * 128]),
                            rhs=_mm(x_cm[:, cc, qt * QT : (qt + 1) * QT]),
                            start=(cc == 0),
                            stop=(cc == CC - 1),
                        )
                    if qt % 2 == 0:
                        nc.scalar.activation(
                            out=_rw(qkvT[:, m, qt * QT : (qt + 1) * QT]),
                            in_=ps,
                            func=mybir.ActivationFunctionType.Identity,
                            bias=bias2[:, m : m + 1],
                        )
                    else:
                        nc.vector.tensor_scalar_add(
                            out=_rw(qkvT[:, m, qt * QT : (qt + 1) * QT]),
                            in0=ps,
                            scalar1=bias2[:, m : m + 1],
                        )

        # ---- phase C: V token-major via PE transposes ----
        with tc.tile_pool(name="vtm", bufs=1) as vtm_pool:
            v_tm = vtm_pool.tile([128, 32, C], F32)
            for nt in range(32):
                for cc in range(CC):
                    ps = psum_tr.tile([128, 128], F32, tag="tr")
                    nc.tensor.transpose(
                        ps, qkvT[:, 4 + cc, nt * 128 : (nt + 1) * 128], ident
                    )
                    # alternate drains across DVE and ACT: the DVE-only chain
                    # (~29us) outlived phase C and starved the first q tile's
                    # denominator adds
                    if (nt + cc) % 2 == 0:
                        nc.vector.tensor_copy(
                            out=_rw(v_tm[:, nt, cc * 128 : (cc + 1) * 128]),
                            in_=ps,
                        )
                    else:
                        nc.scalar.copy(
                            out=_rw(v_tm[:, nt, cc * 128 : (cc + 1) * 128]),
                            in_=ps,
                        )

            # ---- phase D: attention + proj + skip, per q tile ----
            with (
                tc.tile_pool(name="expp", bufs=6) as expp,
                tc.tile_pool(name="accp", bufs=2) as accp,
                tc.tile_pool(name="owork", bufs=2) as owork,
            ):
                def emit_lg(qt, kt):
                    lg = psum_mm.tile([128, QT], F32, tag="mm", name="lg")
                    for cc in range(CC):
                        nc.tensor.matmul(
                            lg,
                            lhsT=_mm(qkvT[:, 2 + cc, kt * 128 : (kt + 1) * 128]),
                            rhs=_mm(qkvT[:, cc, qt * QT : (qt + 1) * QT]),
                            start=(cc == 0),
                            stop=(cc == CC - 1),
                        )
                    return lg

                # logits tiles prefetched across the qt boundary
                next_lgs = {kk: emit_lg(0, kk) for kk in range(2)}
                for qt in range(NQ):
                    av_ps = [
                        psum_acc.tile(
                            [128, QT], F32, tag=f"av_ps{cc}", name=f"av_ps{cc}"
                        )
                        for cc in range(CC)
                    ]
                    expacc = accp.tile([128, QT], F32, tag="expacc")
                    expacc2 = accp.tile([128, QT], F32, tag="expacc2")

                    def emit_av(kt, expT):
                        for cc in range(CC):
                            nc.tensor.matmul(
                                av_ps[cc],
                                lhsT=_mm(v_tm[:, kt, cc * 128 : (cc + 1) * 128]),
                                rhs=_mm(expT),
                                start=(kt == 0),
                                stop=(kt == NK - 1),
                            )

                    # software pipeline, kt unrolled by 2: the PE stays 4+
                    # logits-matmuls ahead of each av, fully hiding the
                    # lg -> exp(ACT) -> av semaphore+latency chain (~910ns).
                    lgs = next_lgs
                    lgs[2] = emit_lg(qt, 2)
                    lgs[3] = emit_lg(qt, 3)
                    for kt0 in range(0, NK, 2):
                        expTs = {}
                        for j in (kt0, kt0 + 1):
                            lg = lgs.pop(j)
                            expT = expp.tile([128, QT], F32, tag="expT")
                            nc.scalar.activation(
                                out=_rw(expT),
                                in_=lg,
                                func=mybir.ActivationFunctionType.Exp,
                                scale=1.0 / 16.0,
                            )
                            expTs[j] = expT
                            # softmax-denominator accumulation on the DVE,
                            # two alternating accumulators to halve the
                            # serial dependence
                            eng = nc.vector
                            acc = expacc2 if j % 2 == 0 else expacc
                            if j < 2:
                                eng.tensor_copy(out=acc, in_=expT)
                            else:
                                eng.tensor_add(out=acc, in0=acc, in1=expT)
                        for j in (kt0 + 4, kt0 + 5):
                            if j < NK:
                                lgs[j] = emit_lg(qt, j)
                        for j in (kt0, kt0 + 1):
                            emit_av(j, expTs[j])

                    # prefetch the next q tile's first logits so the PE
                    # stays busy while the colsum/proj chain drains
                    if qt + 1 < NQ:
                        next_lgs = {kk: emit_lg(qt + 1, kk) for kk in range(2)}
                    expcomb = accp.tile([128, QT], F32, tag="expcomb")
                    nc.vector.tensor_add(
                        out=_rw(expcomb), in0=expacc, in1=expacc2
                    )
                    # softmax denominator, broadcast to every partition by an
                    # all-ones stationary matrix: cs[p, q] = colsum[q] for all
                    # p. 1/colsum then applies channel-major as a plain
                    # elementwise multiply -- no DRAM bounce, no scatter.
                    cs = psum_mm.tile([128, QT], F32, tag="mm", name="cs")
                    nc.tensor.matmul(
                        cs, lhsT=_mm(ones_mat), rhs=_mm(expcomb), start=True, stop=True
                    )
                    recip_b = owork.tile([128, QT], F32, tag="recip_b")
                    nc.vector.reciprocal(out=recip_b, in_=cs)

                    # proj_un^T = w_proj.T @ o_un^T; then *1/colsum + b_proj
                    av_sb = owork.tile([128, CC, QT], F32, tag="av_sb")
                    nc.vector.tensor_copy(out=_rw(av_sb[:, 0, :]), in_=av_ps[0])
                    nc.scalar.copy(out=_rw(av_sb[:, 1, :]), in_=av_ps[1])
                    pj_sb = owork.tile([128, CC, QT], F32, tag="pj_sb")
                    for dc in range(CC):
                        ps = psum_mm.tile([128, QT], F32, tag="mm", name="pj_ps")
                        for cc in range(CC):
                            nc.tensor.matmul(
                                ps,
                                lhsT=_mm(wp[:, cc, dc * 128 : (dc + 1) * 128]),
                                rhs=_mm(av_sb[:, cc, :]),
                                start=(cc == 0),
                                stop=(cc == CC - 1),
                            )
                        nc.vector.tensor_mul(
                            out=pj_sb[:, dc, :], in0=ps, in1=recip_b
                        )
                        nc.scalar.add(
                            out=pj_sb[:, dc, :],
                            in_=pj_sb[:, dc, :],
                            add=bp_col[:, dc : dc + 1],
                        )

                    # back to token-major; add skip
                    out_sb = owork.tile([128, 4, C], F32, tag="out_sb")
                    x_re = owork.tile([128, 4, C], F32, tag="x_re")
                    nc.sync.dma_start(
                        out=x_re, in_=x_tok[:, qt * 4 : (qt + 1) * 4, :]
                    )
                    for qq in range(4):
                        for dc in range(CC):
                            ps = psum_tr.tile([128, 128], F32, tag="tr", name="ps_out")
                            nc.tensor.transpose(
                                ps, pj_sb[:, dc, qq * 128 : (qq + 1) * 128], ident
                            )
                            nc.scalar.copy(
                                out=out_sb[:, qq, dc * 128 : (dc + 1) * 128],
                                in_=ps,
                            )
                    nc.vector.tensor_add(out=out_sb, in0=out_sb, in1=x_re)
                    nc.sync.dma_start(
                        out=out_tok[:, qt * 4 : (qt + 1) * 4, :], in_=out_sb
                    )


_NC = None


def _get_nc():
    global _NC
    if _NC is None:
        _NC = _build()
    return _NC


_RUNNER = None
_ZEROS_FN = None

IN_NAMES = ["x", "gn_scale", "gn_bias", "w_qkv", "b_qkv", "w_proj", "b_proj"]


def _get_runner():
    """Cached jitted shard_map executable over the 8 cores (the equivalent of
    run_bass_kernel_spmd's axon path, but built once instead of per call)."""
    global _RUNNER
    if _RUNNER is not None:
        return _RUNNER
    import jax
    from jax.sharding import Mesh, PartitionSpec
    from jax.experimental.shard_map import shard_map
    from concourse import bass2jax

    nc = _get_nc()
    bass2jax.install_neuronx_cc_hook()

    in_names = list(IN_NAMES) + ["out"]
    if nc.partition_id_tensor is not None:
        in_names.append(nc.partition_id_tensor.name)

    def _body_fn(*args):
        operands = list(args)
        if nc.partition_id_tensor is not None:
            operands.append(bass2jax.partition_id_tensor())
        outs = bass2jax._bass_exec_p.bind(
            *operands,
            out_avals=(jax.core.ShapedArray((N, C), np.float32),),
            in_names=tuple(in_names),
            out_names=("out",),
            lowering_input_output_aliases=(),
            sim_require_finite=True,
            sim_require_nnan=True,
            nc=nc,
        )
        return tuple(outs)

    devices = jax.devices()[:N_CORES]
    mesh = Mesh(np.asarray(devices), ("core",))
    in_specs = (PartitionSpec("core"),) * (len(IN_NAMES) + 1)
    out_specs = (PartitionSpec("core"),)
    sharded = jax.jit(
        shard_map(
            _body_fn, mesh=mesh, in_specs=in_specs, out_specs=out_specs,
            check_rep=False,
        ),
        donate_argnums=(len(IN_NAMES),),
        keep_unused=True,
    )
    _RUNNER = sharded
    return _RUNNER


def kernel(x, gn_scale, gn_bias, w_qkv, b_qkv, w_proj, b_proj):
    sharded = _get_runner()
    x = np.ascontiguousarray(np.asarray(x, dtype=np.float32).reshape(B * N, C))
    shared = {
        "gn_scale": np.asarray(gn_scale, np.float32),
        "gn_bias": np.asarray(gn_bias, np.float32),
        "w_qkv": np.ascontiguousarray(np.asarray(w_qkv, np.float32)),
        "b_qkv": np.asarray(b_qkv, np.float32),
        "w_proj": np.ascontiguousarray(np.asarray(w_proj, np.float32)),
        "b_proj": np.asarray(b_proj, np.float32),
    }
    # shard_map slices axis 0 across cores: x gets its own batch; the shared
    # weights are tiled 8x so every core sees an identical copy.
    concat = [x]
    for name in IN_NAMES[1:]:
        a = shared[name]
        concat.append(np.concatenate([a] * N_CORES, axis=0))
    # donated output buffer, created on-device (saves a 32MB host->device
    # transfer through the axon tunnel every call)
    import jax
    import jax.numpy as jnp
    from jax.sharding import Mesh, NamedSharding, PartitionSpec

    global _ZEROS_FN
    if _ZEROS_FN is None:
        mesh = Mesh(np.asarray(jax.devices()[:N_CORES]), ("core",))
        sh = NamedSharding(mesh, PartitionSpec("core"))
        _ZEROS_FN = jax.jit(
            lambda: jnp.zeros((N_CORES * N, C), jnp.float32), out_shardings=sh
        )
    zeros = _ZEROS_FN()
    (out,) = sharded(*concat, zeros)
    return np.asarray(out).reshape(B, H, W, C)



# revision 10
# speedup vs baseline: 1.7929x; 1.7929x over previous
"""AttentionBlockWithSkipConnection Trainium2 kernel.

Full inputs -> full output. Data-parallel over batch B=8 across 8 cores.
Each core computes one batch: GroupNorm -> qkv 1x1conv -> full 4096x4096
attention -> proj 1x1conv -> skip add.

v2 layout/precision strategy (channel-major middle, mixed precision):
  x^T [C, N] bf16       (64 PE transposes of the fp32 x, drains convert)
  GroupNorm folded into the qkv weights (bf16): qkv^T = (w*a)^T x^T + bias2
  qkvT [128, 6, N] fp8e4   (q,k,v all quantized once on the bias-add drain)
  logits^T[k,q] = K.T @ Q  as ONE DoubleRow fp8 matmul per (qt, kt): the
      [128, 2, *] pair dim covers the full C=256 contraction at 0.5 cyc/row
  expT = exp(logits/16 - 4) -> fp8e4 (shift keeps exp <= e^3.5, well inside
      fp8e4 range; the shift cancels exactly in the softmax quotient).
      One ACT op per kt PAIR ([128, 2, 512] spanning 2 PSUM banks).
  denominator = ones[128,2,1] DoubleRow matmul accumulated over kt pairs
      -> colsum [1, 512] on PE (frees the DVE from 21us/qt of adds)
  o_un^T = V.T @ expT   (DoubleRow fp8, V token-major via 64 fp8 transposes)
  recip = 1/colsum broadcast to 128 partitions by a K=1 ones matmul
  av_sb = o_un^T * recip (bf16)  -> proj TOKEN-major: out[tok,c] via
      lhsT=av_sb chunks (bf16, FWL) -- no output transposes needed
  out = proj + b_proj + x  (DVE adds, fp32 skip from resident x_tm)

PSUM budget (16KB/partition): lg2 pair 4KB + den 2KB + rec 2KB + pj 4KB +
av0/av1 4KB = 16KB exactly; prologue pools are scoped and released first.
"""

import numpy as np

import concourse.bacc as bacc
import concourse.mybir as mybir
import concourse.tile as tile

N_CORES = 8
B, H, W, C = 8, 64, 64, 256
N = H * W  # 4096 tokens
G = 32  # groups
GS = C // G  # 8 channels per group
EPS = 1e-5
CC = C // 128  # 2 channel chunks
QT = 512  # q tile (free dim of logits/attnv matmuls)
NQ = N // QT  # 8
NK = N // 128  # 32 k tiles
NP = NK // 2  # 16 k-tile pairs (DoubleRow)
F32 = mybir.dt.float32
BF16 = mybir.dt.bfloat16
F8 = mybir.dt.float8e4
DRM = mybir.MatmulPerfMode.DoubleRow
AF = mybir.ActivationFunctionType
EXP_SHIFT = 4.0


def _build(repeat=1):
    nc = bacc.Bacc(
        "TRN2",
        target_bir_lowering=False,
        debug=False,
        enable_asserts=True,
        num_devices=N_CORES,
    )
    x_d = nc.dram_tensor("x", [N, C], F32, kind="ExternalInput")
    gns_d = nc.dram_tensor("gn_scale", [C], F32, kind="ExternalInput")
    gnb_d = nc.dram_tensor("gn_bias", [C], F32, kind="ExternalInput")
    wq_d = nc.dram_tensor("w_qkv", [C, 3 * C], F32, kind="ExternalInput")
    bq_d = nc.dram_tensor("b_qkv", [3 * C], F32, kind="ExternalInput")
    wp_d = nc.dram_tensor("w_proj", [C, C], F32, kind="ExternalInput")
    bp_d = nc.dram_tensor("b_proj", [C], F32, kind="ExternalInput")
    out_d = nc.dram_tensor("out", [N, C], F32, kind="ExternalOutput")

    # group-aggregation masks: gA averages 8 consecutive partitions into one
    # group row; gB broadcasts group rows back to their 128 channels.
    gA_np = np.zeros((128, 16), np.float32)
    gB_np = np.zeros((16, 128), np.float32)
    for p in range(128):
        gA_np[p, p // GS] = 1.0 / GS
        gB_np[p // GS, p] = 1.0
    gA_d = nc.inline_tensor(gA_np, "gA")
    gB_d = nc.inline_tensor(gB_np, "gB")
    ident_d = nc.inline_tensor(np.eye(128, dtype=np.float32), "ident")

    with tile.TileContext(nc) as tc:
        for _ in range(repeat):
            _body(tc, x_d, gns_d, gnb_d, wq_d, bq_d, wp_d, bp_d, out_d,
                  gA_d, gB_d, ident_d)
    nc.compile()
    return nc


def _body(tc, x_d, gns_d, gnb_d, wq_d, bq_d, wp_d, bp_d, out_d,
          gA_d, gB_d, ident_d):
    nc = tc.nc
    x_tok = x_d.ap().rearrange("(p nt) c -> p nt c", p=128)  # [128, 32, 256]
    out_tok = out_d.ap().rearrange("(p nt) c -> p nt c", p=128)

    with (
        nc.allow_low_precision("mixed-precision attention: bf16/fp8 matmul "
                               "operands, fp32 accumulation throughout"),
        tc.tile_pool(name="consts", bufs=1) as consts,
        tc.tile_pool(name="xtm", bufs=2) as xtm_pool,
        tc.tile_pool(name="xcm", bufs=1) as xcm_pool,
        tc.tile_pool(name="qkvT", bufs=1) as qkvT_pool,
        tc.tile_pool(name="vtm", bufs=1) as vtm_pool,
    ):
        # ---- input DMAs: x first (PE transposes gate on it) ----
        ident = consts.tile([128, 128], F32)
        nc.sync.dma_start(out=ident, in_=ident_d.ap())
        x_tm = xtm_pool.tile([128, 32, C], F32, tag="x_tm")  # 32KB/partition
        dma_engs = [nc.sync, nc.scalar]
        for dchunk in range(16):
            dma_engs[dchunk % 2].dma_start(
                out=x_tm[:, dchunk * 2 : (dchunk + 1) * 2, :],
                in_=x_tok[:, dchunk * 2 : (dchunk + 1) * 2, :],
            )

        # ---- weights / small constants behind the x chunks ----
        gA = consts.tile([128, 16], F32)
        nc.sync.dma_start(out=gA, in_=gA_d.ap())
        gB = consts.tile([16, 128], F32)
        nc.scalar.dma_start(out=gB, in_=gB_d.ap())
        wq_stage = consts.tile([128, CC, 3 * C], F32)
        nc.scalar.dma_start(
            out=wq_stage, in_=wq_d.ap().rearrange("(cc p) d -> p cc d", p=128)
        )
        wp_stage = consts.tile([128, CC, C], F32)
        nc.sync.dma_start(
            out=wp_stage, in_=wp_d.ap().rearrange("(cc p) d -> p cc d", p=128)
        )
        wp_bf = consts.tile([128, CC, C], BF16)
        nc.vector.tensor_copy(out=wp_bf, in_=wp_stage)
        bq = consts.tile([128, 6], F32)
        nc.sync.dma_start(
            out=bq, in_=bq_d.ap().rearrange("(m p) -> p m", p=128)
        )
        bp_stage = consts.tile([1, C], F32)
        nc.sync.dma_start(
            out=bp_stage, in_=bp_d.ap().rearrange("(a c) -> a c", a=1)
        )
        gns = consts.tile([128, CC], F32)
        nc.scalar.dma_start(
            out=gns, in_=gns_d.ap().rearrange("(cc p) -> p cc", p=128)
        )
        gnb = consts.tile([128, CC], F32)
        nc.sync.dma_start(
            out=gnb, in_=gnb_d.ap().rearrange("(cc p) -> p cc", p=128)
        )
        ones_raw = consts.tile([128, 128], F32)
        nc.vector.memset(ones_raw, 1.0)
        ident8 = consts.tile([128, 128], F8)
        nc.vector.tensor_copy(out=ident8, in_=ident)
        # denominator DR stationary: [128, 2, 16] so the pair-dim stride is
        # 16 bytes (DoubleRow LDWEIGHTS requires step % 16 == 0); only
        # [:, :, 0:1] is used as the weights column.
        ones8_t = consts.tile([128, 2, 16], F8)
        nc.vector.tensor_copy(out=ones8_t, in_=ones_raw[:, 0:32])
        ones8 = ones8_t[:, :, 0:1]
        ones_col_bf = consts.tile([1, 128], BF16)  # K=1 broadcast stationary
        nc.vector.tensor_copy(out=ones_col_bf, in_=ones_raw[0:1, :])
        ones_col_f = consts.tile([1, 128], F32)
        nc.vector.tensor_copy(out=ones_col_f, in_=ones_raw[0:1, :])
        eps_col = consts.tile([128, 1], F32)
        nc.vector.memset(eps_col, EPS)
        nshift_col = consts.tile([128, 1], F32)
        nc.vector.memset(nshift_col, -EXP_SHIFT)

        x_cm = xcm_pool.tile([128, CC, N], BF16, tag="x_cm")  # 16KB/partition
        qkvT = qkvT_pool.tile([128, 6, N], F8, tag="qkvT")  # 24KB/partition
        v_tm = vtm_pool.tile([128, NK, C], F8, tag="v_tm")  # 8KB/partition
        wq_bf = consts.tile([128, CC, 3 * C], BF16)  # folded qkv weights
        bp4 = consts.tile([128, 4, C], F32)  # b_proj broadcast 128x4 rows

        with (
            tc.tile_pool(name="pro_ps", bufs=2, space="PSUM") as pro_ps,
            tc.tile_pool(name="pro_mm", bufs=2, space="PSUM") as pro_mm,
            tc.tile_pool(name="gn_stats", bufs=2) as gn_stats,
        ):
            # b_proj row -> [128, 256] broadcast (once), then 4 row copies
            bp_ps = pro_mm.tile([128, C], F32, tag="bp_ps", name="bp_ps", bufs=1)
            nc.tensor.matmul(
                bp_ps, lhsT=ones_col_f, rhs=bp_stage, start=True, stop=True
            )
            for r in range(4):
                nc.vector.tensor_copy(out=bp4[:, r, :], in_=bp_ps)

            # ---- phase A: transpose x to channel-major bf16; bn_stats
            # interleaved so statistics finish right after the last chunk ----
            stats = gn_stats.tile([128, CC, 8, 6], F32)
            for s in range(8):
                for nt in range(4 * s, 4 * s + 4):
                    for cc in range(CC):
                        ps = pro_ps.tile([128, 128], F32, tag="trx")
                        nc.tensor.transpose(
                            ps, x_tm[:, nt, cc * 128 : (cc + 1) * 128], ident
                        )
                        if (nt + cc) % 2 == 0:
                            nc.vector.tensor_copy(
                                out=x_cm[:, cc, nt * 128 : (nt + 1) * 128],
                                in_=ps,
                            )
                        else:
                            nc.scalar.copy(
                                out=x_cm[:, cc, nt * 128 : (nt + 1) * 128],
                                in_=ps,
                            )
                for cc in range(CC):
                    nc.vector.bn_stats(
                        out=stats[:, cc, s, :],
                        in_=x_cm[:, cc, s * 512 : (s + 1) * 512],
                    )

            # ---- groupnorm stats -> per-channel affine (a, b) ----
            ab = gn_stats.tile([128, CC, 2], F32)  # (a, b) per channel
            for cc in range(CC):
                mv = gn_stats.tile([128, 2], F32, tag="mv")
                nc.vector.bn_aggr(out=mv, in_=stats[:, cc, :, :])
                mv2 = gn_stats.tile([128, 2], F32, tag="mv2")
                nc.vector.tensor_copy(out=mv2[:, 0:1], in_=mv[:, 0:1])
                nc.vector.tensor_mul(out=mv2[:, 1:2], in0=mv[:, 0:1], in1=mv[:, 0:1])
                nc.vector.tensor_add(out=mv2[:, 1:2], in0=mv2[:, 1:2], in1=mv[:, 1:2])
                gp = pro_ps.tile([128, 2], F32, tag="smm", name="gp", bufs=1)[0:16, :]
                nc.tensor.matmul(gp, lhsT=gA, rhs=mv2, start=True, stop=True)
                gp_sb = gn_stats.tile([16, 2], F32, tag="gp_sb")
                nc.vector.tensor_copy(out=gp_sb, in_=gp)
                chs = pro_ps.tile([128, 2], F32, tag="smm", name="chs", bufs=1)
                nc.tensor.matmul(chs, lhsT=gB, rhs=gp_sb, start=True, stop=True)
                chs_sb = gn_stats.tile([128, 2], F32, tag="chs_sb")
                nc.vector.tensor_copy(out=chs_sb, in_=chs)
                var = gn_stats.tile([128, 1], F32, tag="var")
                msq = gn_stats.tile([128, 1], F32, tag="msq")
                nc.vector.tensor_mul(out=msq, in0=chs_sb[:, 0:1], in1=chs_sb[:, 0:1])
                nc.vector.tensor_sub(out=var, in0=chs_sb[:, 1:2], in1=msq)
                nc.scalar.activation(
                    out=var, in_=var, func=AF.Sqrt, bias=eps_col
                )
                rstd = gn_stats.tile([128, 1], F32, tag="rstd")
                nc.vector.reciprocal(out=rstd, in_=var)
                nc.vector.tensor_mul(
                    out=ab[:, cc, 0:1], in0=rstd, in1=gns[:, cc : cc + 1]
                )
                nc.vector.tensor_mul(out=msq, in0=chs_sb[:, 0:1], in1=ab[:, cc, 0:1])
                nc.vector.tensor_sub(
                    out=ab[:, cc, 1:2], in0=gnb[:, cc : cc + 1], in1=msq
                )

            # ---- fold the affine into the qkv weights (bf16):
            # qkv^T = (w*a)^T x^T + (w^T b + b_qkv) ----
            for m in range(6):
                for cc in range(CC):
                    nc.scalar.mul(
                        out=wq_bf[:, cc, m * 128 : (m + 1) * 128],
                        in_=wq_stage[:, cc, m * 128 : (m + 1) * 128],
                        mul=ab[:, cc, 0:1],
                    )
            bias2 = gn_stats.tile([128, 6], F32)
            for m in range(6):
                psb = pro_ps.tile([128, 2], F32, tag="smm", name="psb", bufs=1)[:, 0:1]
                for cc in range(CC):
                    nc.tensor.matmul(
                        psb,
                        lhsT=wq_stage[:, cc, m * 128 : (m + 1) * 128],
                        rhs=ab[:, cc, 1:2],
                        start=(cc == 0),
                        stop=(cc == CC - 1),
                    )
                nc.vector.tensor_add(
                    out=bias2[:, m : m + 1], in0=psb, in1=bq[:, m : m + 1]
                )

            # ---- phase B: qkv^T = wq.T @ x^T (+ bias2), bf16 in, fp8 out ----
            for m in range(6):
                for qt in range(NQ):
                    ps = pro_mm.tile([128, QT], F32, tag="qkv")
                    for cc in range(CC):
                        nc.tensor.matmul(
                            ps,
                            lhsT=wq_bf[:, cc, m * 128 : (m + 1) * 128],
                            rhs=x_cm[:, cc, qt * QT : (qt + 1) * QT],
                            start=(cc == 0),
                            stop=(cc == CC - 1),
                        )
                    if qt % 2 == 0:
                        nc.scalar.activation(
                            out=qkvT[:, m, qt * QT : (qt + 1) * QT],
                            in_=ps,
                            func=AF.Identity,
                            bias=bias2[:, m : m + 1],
                        )
                    else:
                        nc.vector.tensor_scalar_add(
                            out=qkvT[:, m, qt * QT : (qt + 1) * QT],
                            in0=ps,
                            scalar1=bias2[:, m : m + 1],
                        )

            # ---- phase C: V token-major via fp8 PE transposes ----
            for nt in range(NK):
                for cc in range(CC):
                    # fp8 transpose writes PSUM in 2-byte granules: the
                    # verifier requires output element step 2, so view the
                    # tile as [128, 128, 2] and write plane 0.
                    ps8 = pro_ps.tile([128, 128, 2], F8, tag="trv", name="ps8")[:, :, 0]
                    nc.tensor.transpose(
                        ps8, qkvT[:, 4 + cc, nt * 128 : (nt + 1) * 128], ident8
                    )
                    if (nt + cc) % 2 == 0:
                        nc.vector.tensor_copy(
                            out=v_tm[:, nt, cc * 128 : (cc + 1) * 128],
                            in_=ps8,
                        )
                    else:
                        nc.scalar.copy(
                            out=v_tm[:, nt, cc * 128 : (cc + 1) * 128],
                            in_=ps8,
                        )

        # ---- phase D: attention + proj + skip, per q tile ----
        with (
            tc.tile_pool(name="lgp", bufs=2, space="PSUM") as lgp,
            tc.tile_pool(name="pmisc", bufs=1, space="PSUM") as pmisc,
            tc.tile_pool(name="avp", bufs=1, space="PSUM") as avp,
            tc.tile_pool(name="expp", bufs=3) as expp,
            tc.tile_pool(name="owork", bufs=2) as owork,
        ):
            def emit_lg(qt, kt):
                """One [128, 512] logits tile: a single DoubleRow matmul
                contracting the full C=256 via the fp8 pair dim. Two rotating
                PSUM banks keep the PE ~2 tiles ahead of the ACT exp stream."""
                lg = lgp.tile([128, QT], F32, tag="lg", name="lg")
                nc.tensor.matmul(
                    lg,
                    lhsT=qkvT[:, 2:4, kt * 128 : (kt + 1) * 128],
                    rhs=qkvT[:, 0:2, qt * QT : (qt + 1) * QT],
                    start=True,
                    stop=True,
                    perf_mode=DRM,
                )
                return lg

            def next_lg(qt, kt):
                if kt < NK:
                    return emit_lg(qt, kt)
                if qt + 1 < NQ:
                    return emit_lg(qt + 1, kt - NK)
                return None

            lg_a = emit_lg(0, 0)
            lg_b = emit_lg(0, 1)
            for qt in range(NQ):
                av_ps = [
                    avp.tile([128, QT], F32, tag=f"av{cc}", name=f"av{cc}")
                    for cc in range(CC)
                ]
                den = pmisc.tile([1, QT], F32, tag="den", name="den")

                for pair in range(NP):
                    expT2 = expp.tile([128, 2, QT], F8, tag="expT2",
                                      name="expT2")
                    # exp slot by slot; the -4 shift keeps the fp8 numerator
                    # in range and cancels in the softmax quotient.
                    nc.scalar.activation(
                        out=expT2[:, 0, :],
                        in_=lg_a,
                        func=AF.Exp,
                        scale=1.0 / 16.0,
                        bias=nshift_col,
                    )
                    lg_a = next_lg(qt, 2 * pair + 2)
                    nc.scalar.activation(
                        out=expT2[:, 1, :],
                        in_=lg_b,
                        func=AF.Exp,
                        scale=1.0 / 16.0,
                        bias=nshift_col,
                    )
                    lg_b = next_lg(qt, 2 * pair + 3)
                    for cc in range(CC):
                        nc.tensor.matmul(
                            av_ps[cc],
                            lhsT=v_tm[:, 2 * pair : 2 * pair + 2,
                                      cc * 128 : (cc + 1) * 128],
                            rhs=expT2,
                            start=(pair == 0),
                            stop=(pair == NP - 1),
                            perf_mode=DRM,
                        )
                    nc.tensor.matmul(
                        den,
                        lhsT=ones8,
                        rhs=expT2,
                        start=(pair == 0),
                        stop=(pair == NP - 1),
                        perf_mode=DRM,
                    )

                # ---- tail: softmax denominator -> normalize -> proj ----
                recip_row = owork.tile([1, QT], BF16, tag="recip_row")
                nc.vector.reciprocal(out=recip_row, in_=den)
                rec_ps = pmisc.tile([128, QT], F32, tag="rec", name="rec")
                nc.tensor.matmul(
                    rec_ps, lhsT=ones_col_bf, rhs=recip_row,
                    start=True, stop=True,
                )
                rec_sb = owork.tile([128, QT], BF16, tag="rec_sb")
                nc.vector.tensor_copy(out=rec_sb, in_=rec_ps)
                av_sb = owork.tile([128, CC, QT], BF16, tag="av_sb")
                for cc in range(CC):
                    nc.vector.tensor_mul(
                        out=av_sb[:, cc, :], in0=av_ps[cc], in1=rec_sb
                    )

                # proj token-major: out[tok, c], no output transposes
                pj = pmisc.tile([128, 4, C], F32, tag="pj", name="pj")
                for tt in range(4):
                    for cc in range(CC):
                        nc.tensor.matmul(
                            pj[:, tt, :],
                            lhsT=av_sb[:, cc, tt * 128 : (tt + 1) * 128],
                            rhs=wp_bf[:, cc, :],
                            start=(cc == 0),
                            stop=(cc == CC - 1),
                        )

                out_sb = owork.tile([128, 4, C], F32, tag="out_sb")
                for half in range(2):
                    nc.vector.tensor_add(
                        out=out_sb[:, half * 2 : (half + 1) * 2, :],
                        in0=pj[:, half * 2 : (half + 1) * 2, :],
                        in1=x_tm[:, qt * 4 + half * 2 : qt * 4 + (half + 1) * 2, :],
                    )
                nc.vector.tensor_add(out=out_sb, in0=out_sb, in1=bp4)
                nc.sync.dma_start(
                    out=out_tok[:, qt * 4 : (qt + 1) * 4, :], in_=out_sb
                )


_NC = None


def _get_nc():
    global _NC
    if _NC is None:
        _NC = _build()
    return _NC


_RUNNER = None
_ZEROS_FN = None

IN_NAMES = ["x", "gn_scale", "gn_bias", "w_qkv", "b_qkv", "w_proj", "b_proj"]


def _get_runner():
    """Cached jitted shard_map executable over the 8 cores (the equivalent of
    run_bass_kernel_spmd's axon path, but built once instead of per call)."""
    global _RUNNER
    if _RUNNER is not None:
        return _RUNNER
    import jax
    from jax.sharding import Mesh, PartitionSpec
    from jax.experimental.shard_map import shard_map
    from concourse import bass2jax

    nc = _get_nc()
    bass2jax.install_neuronx_cc_hook()

    in_names = list(IN_NAMES) + ["out"]
    if nc.partition_id_tensor is not None:
        in_names.append(nc.partition_id_tensor.name)

    def _body_fn(*args):
        operands = list(args)
        if nc.partition_id_tensor is not None:
            operands.append(bass2jax.partition_id_tensor())
        outs = bass2jax._bass_exec_p.bind(
            *operands,
            out_avals=(jax.core.ShapedArray((N, C), np.float32),),
            in_names=tuple(in_names),
            out_names=("out",),
            lowering_input_output_aliases=(),
            sim_require_finite=True,
            sim_require_nnan=True,
            nc=nc,
        )
        return tuple(outs)

    devices = jax.devices()[:N_CORES]
    mesh = Mesh(np.asarray(devices), ("core",))
    in_specs = (PartitionSpec("core"),) * (len(IN_NAMES) + 1)
    out_specs = (PartitionSpec("core"),)
    sharded = jax.jit(
        shard_map(
            _body_fn, mesh=mesh, in_specs=in_specs, out_specs=out_specs,
            check_rep=False,
        ),
        donate_argnums=(len(IN_NAMES),),
        keep_unused=True,
    )
    _RUNNER = sharded
    return _RUNNER


def kernel(x, gn_scale, gn_bias, w_qkv, b_qkv, w_proj, b_proj):
    sharded = _get_runner()
    x = np.ascontiguousarray(np.asarray(x, dtype=np.float32).reshape(B * N, C))
    shared = {
        "gn_scale": np.asarray(gn_scale, np.float32),
        "gn_bias": np.asarray(gn_bias, np.float32),
        "w_qkv": np.ascontiguousarray(np.asarray(w_qkv, np.float32)),
        "b_qkv": np.asarray(b_qkv, np.float32),
        "w_proj": np.ascontiguousarray(np.asarray(w_proj, np.float32)),
        "b_proj": np.asarray(b_proj, np.float32),
    }
    # shard_map slices axis 0 across cores: x gets its own batch; the shared
    # weights are tiled 8x so every core sees an identical copy.
    concat = [x]
    for name in IN_NAMES[1:]:
        a = shared[name]
        concat.append(np.concatenate([a] * N_CORES, axis=0))
    # donated output buffer, created on-device (saves a 32MB host->device
    # transfer through the axon tunnel every call)
    import jax
    import jax.numpy as jnp
    from jax.sharding import Mesh, NamedSharding, PartitionSpec

    global _ZEROS_FN
    if _ZEROS_FN is None:
        mesh = Mesh(np.asarray(jax.devices()[:N_CORES]), ("core",))
        sh = NamedSharding(mesh, PartitionSpec("core"))
        _ZEROS_FN = jax.jit(
            lambda: jnp.zeros((N_CORES * N, C), jnp.float32), out_shardings=sh
        )
    zeros = _ZEROS_FN()
    (out,) = sharded(*concat, zeros)
    return np.asarray(out).reshape(B, H, W, C)


# revision 13
# speedup vs baseline: 2.6327x; 1.4684x over previous
"""AttentionBlockWithSkipConnection Trainium2 kernel.

Full inputs -> full output. Data-parallel over batch B=8 across 8 cores.
Each core computes one batch: GroupNorm -> qkv 1x1conv -> full 4096x4096
attention -> proj 1x1conv -> skip add.

v2 layout/precision strategy (channel-major middle, mixed precision):
  x^T [C, N] bf16       (64 PE transposes of the fp32 x, drains convert)
  GroupNorm folded into the qkv weights (bf16): qkv^T = (w*a)^T x^T + bias2
  qkvT [128, 6, N] fp8e4   (q,k,v all quantized once on the bias-add drain)
  logits^T[k,q] = K.T @ Q  as ONE DoubleRow fp8 matmul per (qt, kt): the
      [128, 2, *] pair dim covers the full C=256 contraction at 0.5 cyc/row
  expT = exp(logits/16 - 4) -> fp8e4 (shift keeps exp <= e^3.5, well inside
      fp8e4 range; the shift cancels exactly in the softmax quotient).
      One ACT op per kt PAIR ([128, 2, 512] spanning 2 PSUM banks).
  denominator = ones[128,2,1] DoubleRow matmul accumulated over kt pairs
      -> colsum [1, 512] on PE (frees the DVE from 21us/qt of adds)
  o_un^T = V.T @ expT   (DoubleRow fp8, V token-major via 64 fp8 transposes)
  recip = 1/colsum broadcast to 128 partitions by a K=1 ones matmul
  av_sb = o_un^T * recip (bf16)  -> proj TOKEN-major: out[tok,c] via
      lhsT=av_sb chunks (bf16, FWL) -- no output transposes needed
  out = proj + b_proj + x  (DVE adds, fp32 skip from resident x_tm)

PSUM budget (16KB/partition): lg2 pair 4KB + den 2KB + rec 2KB + pj 4KB +
av0/av1 4KB = 16KB exactly; prologue pools are scoped and released first.
"""

import numpy as np

import concourse.bacc as bacc
import concourse.mybir as mybir
import concourse.tile as tile

N_CORES = 8
B, H, W, C = 8, 64, 64, 256
N = H * W  # 4096 tokens
G = 32  # groups
GS = C // G  # 8 channels per group
EPS = 1e-5
CC = C // 128  # 2 channel chunks
QT = 512  # q tile (free dim of logits/attnv matmuls)
NQ = N // QT  # 8
NK = N // 128  # 32 k tiles
NP = NK // 2  # 16 k-tile pairs (DoubleRow)
F32 = mybir.dt.float32
BF16 = mybir.dt.bfloat16
F8 = mybir.dt.float8e4
DRM = mybir.MatmulPerfMode.DoubleRow
AF = mybir.ActivationFunctionType
EXP_SHIFT = 4.0


def _build(repeat=1):
    nc = bacc.Bacc(
        "TRN2",
        target_bir_lowering=False,
        debug=False,
        enable_asserts=True,
        num_devices=N_CORES,
    )
    x_d = nc.dram_tensor("x", [N, C], F32, kind="ExternalInput")
    gns_d = nc.dram_tensor("gn_scale", [C], F32, kind="ExternalInput")
    gnb_d = nc.dram_tensor("gn_bias", [C], F32, kind="ExternalInput")
    wq_d = nc.dram_tensor("w_qkv", [C, 3 * C], F32, kind="ExternalInput")
    bq_d = nc.dram_tensor("b_qkv", [3 * C], F32, kind="ExternalInput")
    wp_d = nc.dram_tensor("w_proj", [C, C], F32, kind="ExternalInput")
    bp_d = nc.dram_tensor("b_proj", [C], F32, kind="ExternalInput")
    out_d = nc.dram_tensor("out", [N, C], F32, kind="ExternalOutput")

    # group-aggregation masks: gA averages 8 consecutive partitions into one
    # group row; gB broadcasts group rows back to their 128 channels.
    gA_np = np.zeros((128, 16), np.float32)
    gB_np = np.zeros((16, 128), np.float32)
    for p in range(128):
        gA_np[p, p // GS] = 1.0 / GS
        gB_np[p // GS, p] = 1.0
    gA_d = nc.inline_tensor(gA_np, "gA")
    gB_d = nc.inline_tensor(gB_np, "gB")
    ident_d = nc.inline_tensor(np.eye(128, dtype=np.float32), "ident")

    with tile.TileContext(nc) as tc:
        for _ in range(repeat):
            _body(tc, x_d, gns_d, gnb_d, wq_d, bq_d, wp_d, bp_d, out_d,
                  gA_d, gB_d, ident_d)
    nc.compile()
    return nc


def _body(tc, x_d, gns_d, gnb_d, wq_d, bq_d, wp_d, bp_d, out_d,
          gA_d, gB_d, ident_d):
    nc = tc.nc
    x_tok = x_d.ap().rearrange("(p nt) c -> p nt c", p=128)  # [128, 32, 256]
    out_tok = out_d.ap().rearrange("(p nt) c -> p nt c", p=128)

    with (
        nc.allow_low_precision("mixed-precision attention: bf16/fp8 matmul "
                               "operands, fp32 accumulation throughout"),
        tc.tile_pool(name="consts", bufs=1) as consts,
        tc.tile_pool(name="xtm", bufs=2) as xtm_pool,
        tc.tile_pool(name="xcm", bufs=1) as xcm_pool,
        tc.tile_pool(name="qkvT", bufs=1) as qkvT_pool,
        tc.tile_pool(name="vtm", bufs=1) as vtm_pool,
    ):
        # ---- input DMAs: x first (PE transposes gate on it) ----
        ident = consts.tile([128, 128], F32)
        nc.sync.dma_start(out=ident, in_=ident_d.ap())
        x_tm = xtm_pool.tile([128, 32, C], F32, tag="x_tm")  # 32KB/partition
        dma_engs = [nc.sync, nc.scalar]
        for dchunk in range(16):
            dma_engs[dchunk % 2].dma_start(
                out=x_tm[:, dchunk * 2 : (dchunk + 1) * 2, :],
                in_=x_tok[:, dchunk * 2 : (dchunk + 1) * 2, :],
            )

        # ---- weights / small constants behind the x chunks ----
        gA = consts.tile([128, 16], F32)
        nc.sync.dma_start(out=gA, in_=gA_d.ap())
        gB = consts.tile([16, 128], F32)
        nc.scalar.dma_start(out=gB, in_=gB_d.ap())
        wq_stage = consts.tile([128, CC, 3 * C], F32)
        nc.scalar.dma_start(
            out=wq_stage, in_=wq_d.ap().rearrange("(cc p) d -> p cc d", p=128)
        )
        wp_stage = consts.tile([128, CC, C], F32)
        nc.sync.dma_start(
            out=wp_stage, in_=wp_d.ap().rearrange("(cc p) d -> p cc d", p=128)
        )
        wp_bf = consts.tile([128, CC, C], BF16)
        nc.vector.tensor_copy(out=wp_bf, in_=wp_stage)
        bq = consts.tile([128, 6], F32)
        nc.sync.dma_start(
            out=bq, in_=bq_d.ap().rearrange("(m p) -> p m", p=128)
        )
        bp_stage = consts.tile([1, C], F32)
        nc.sync.dma_start(
            out=bp_stage, in_=bp_d.ap().rearrange("(a c) -> a c", a=1)
        )
        gns = consts.tile([128, CC], F32)
        nc.scalar.dma_start(
            out=gns, in_=gns_d.ap().rearrange("(cc p) -> p cc", p=128)
        )
        gnb = consts.tile([128, CC], F32)
        nc.sync.dma_start(
            out=gnb, in_=gnb_d.ap().rearrange("(cc p) -> p cc", p=128)
        )
        ones_raw = consts.tile([128, 128], F32)
        nc.vector.memset(ones_raw, 1.0)
        ident8 = consts.tile([128, 128], F8)
        nc.vector.tensor_copy(out=ident8, in_=ident)
        # denominator DR stationary: [128, 2, 16] so the pair-dim stride is
        # 16 bytes (DoubleRow LDWEIGHTS requires step % 16 == 0); only
        # [:, :, 0:1] is used as the weights column.
        ones8_t = consts.tile([128, 2, 16], F8)
        nc.vector.tensor_copy(out=ones8_t, in_=ones_raw[:, 0:32])
        ones8 = ones8_t[:, :, 0:1]
        ones_col_bf = consts.tile([1, 128], BF16)  # K=1 broadcast stationary
        nc.vector.tensor_copy(out=ones_col_bf, in_=ones_raw[0:1, :])
        ones_col_f = consts.tile([1, 128], F32)
        nc.vector.tensor_copy(out=ones_col_f, in_=ones_raw[0:1, :])
        eps_col = consts.tile([128, 1], F32)
        nc.vector.memset(eps_col, EPS)
        nshift_col = consts.tile([128, 1], F32)
        nc.vector.memset(nshift_col, -EXP_SHIFT)

        x_cm = xcm_pool.tile([128, CC, N], BF16, tag="x_cm")  # 16KB/partition
        qkvT = qkvT_pool.tile([128, 6, N], F8, tag="qkvT")  # 24KB/partition
        v_tm = vtm_pool.tile([128, NK, C], F8, tag="v_tm")  # 8KB/partition
        wq_bf = consts.tile([128, CC, 3 * C], BF16)  # folded qkv weights
        bp4 = consts.tile([128, 4, C], F32)  # b_proj broadcast 128x4 rows

        with (
            tc.tile_pool(name="pro_ps", bufs=2, space="PSUM") as pro_ps,
            tc.tile_pool(name="pro_mm", bufs=2, space="PSUM") as pro_mm,
            tc.tile_pool(name="gn_stats", bufs=2) as gn_stats,
        ):
            # b_proj row -> [128, 256] broadcast (once), then 4 row copies
            bp_ps = pro_mm.tile([128, C], F32, tag="bp_ps", name="bp_ps", bufs=1)
            nc.tensor.matmul(
                bp_ps, lhsT=ones_col_f, rhs=bp_stage, start=True, stop=True
            )
            for r in range(4):
                nc.vector.tensor_copy(out=bp4[:, r, :], in_=bp_ps)

            # ---- phase A: transpose x to channel-major bf16; bn_stats
            # interleaved so statistics finish right after the last chunk ----
            stats = gn_stats.tile([128, CC, 8, 6], F32)
            for s in range(8):
                for nt in range(4 * s, 4 * s + 4):
                    # both cc chunks transpose into one [128, 256] bank so a
                    # single batched drain amortizes the PSUM-read bubble
                    ps = pro_ps.tile([128, 256], F32, tag="trx", name="ps")
                    for cc in range(CC):
                        nc.tensor.transpose(
                            ps[:, cc * 128 : (cc + 1) * 128],
                            x_tm[:, nt, cc * 128 : (cc + 1) * 128],
                            ident,
                        )
                    dst = x_cm[:, 0:CC, nt * 128 : (nt + 1) * 128]
                    if nt % 2 == 0:
                        nc.vector.tensor_copy(out=dst, in_=ps)
                    else:
                        nc.scalar.copy(out=dst, in_=ps)
                for cc in range(CC):
                    nc.vector.bn_stats(
                        out=stats[:, cc, s, :],
                        in_=x_cm[:, cc, s * 512 : (s + 1) * 512],
                    )

            # ---- groupnorm stats -> per-channel affine (a, b) ----
            ab = gn_stats.tile([128, CC, 2], F32)  # (a, b) per channel
            for cc in range(CC):
                mv = gn_stats.tile([128, 2], F32, tag="mv")
                nc.vector.bn_aggr(out=mv, in_=stats[:, cc, :, :])
                mv2 = gn_stats.tile([128, 2], F32, tag="mv2")
                nc.vector.tensor_copy(out=mv2[:, 0:1], in_=mv[:, 0:1])
                nc.vector.tensor_mul(out=mv2[:, 1:2], in0=mv[:, 0:1], in1=mv[:, 0:1])
                nc.vector.tensor_add(out=mv2[:, 1:2], in0=mv2[:, 1:2], in1=mv[:, 1:2])
                gp = pro_ps.tile([128, 2], F32, tag="smm", name="gp", bufs=1)[0:16, :]
                nc.tensor.matmul(gp, lhsT=gA, rhs=mv2, start=True, stop=True)
                gp_sb = gn_stats.tile([16, 2], F32, tag="gp_sb")
                nc.vector.tensor_copy(out=gp_sb, in_=gp)
                chs = pro_ps.tile([128, 2], F32, tag="smm", name="chs", bufs=1)
                nc.tensor.matmul(chs, lhsT=gB, rhs=gp_sb, start=True, stop=True)
                chs_sb = gn_stats.tile([128, 2], F32, tag="chs_sb")
                nc.vector.tensor_copy(out=chs_sb, in_=chs)
                var = gn_stats.tile([128, 1], F32, tag="var")
                msq = gn_stats.tile([128, 1], F32, tag="msq")
                nc.vector.tensor_mul(out=msq, in0=chs_sb[:, 0:1], in1=chs_sb[:, 0:1])
                nc.vector.tensor_sub(out=var, in0=chs_sb[:, 1:2], in1=msq)
                nc.scalar.activation(
                    out=var, in_=var, func=AF.Sqrt, bias=eps_col
                )
                rstd = gn_stats.tile([128, 1], F32, tag="rstd")
                nc.vector.reciprocal(out=rstd, in_=var)
                nc.vector.tensor_mul(
                    out=ab[:, cc, 0:1], in0=rstd, in1=gns[:, cc : cc + 1]
                )
                nc.vector.tensor_mul(out=msq, in0=chs_sb[:, 0:1], in1=ab[:, cc, 0:1])
                nc.vector.tensor_sub(
                    out=ab[:, cc, 1:2], in0=gnb[:, cc : cc + 1], in1=msq
                )

            # ---- fold the affine into the qkv weights (bf16):
            # qkv^T = (w*a)^T x^T + (w^T b + b_qkv) ----
            for m in range(6):
                for cc in range(CC):
                    nc.scalar.mul(
                        out=wq_bf[:, cc, m * 128 : (m + 1) * 128],
                        in_=wq_stage[:, cc, m * 128 : (m + 1) * 128],
                        mul=ab[:, cc, 0:1],
                    )
            bias2 = gn_stats.tile([128, 6], F32)
            for m in range(6):
                psb = pro_ps.tile([128, 2], F32, tag="smm", name="psb", bufs=1)[:, 0:1]
                for cc in range(CC):
                    nc.tensor.matmul(
                        psb,
                        lhsT=wq_stage[:, cc, m * 128 : (m + 1) * 128],
                        rhs=ab[:, cc, 1:2],
                        start=(cc == 0),
                        stop=(cc == CC - 1),
                    )
                nc.vector.tensor_add(
                    out=bias2[:, m : m + 1], in0=psb, in1=bq[:, m : m + 1]
                )

            # ---- phase B: qkv^T = wq.T @ x^T (+ bias2), bf16 in, fp8 out ----
            for m in range(6):
                for qt in range(NQ):
                    ps = pro_mm.tile([128, QT], F32, tag="qkv")
                    for cc in range(CC):
                        nc.tensor.matmul(
                            ps,
                            lhsT=wq_bf[:, cc, m * 128 : (m + 1) * 128],
                            rhs=x_cm[:, cc, qt * QT : (qt + 1) * QT],
                            start=(cc == 0),
                            stop=(cc == CC - 1),
                        )
                    if qt % 2 == 0:
                        nc.scalar.activation(
                            out=qkvT[:, m, qt * QT : (qt + 1) * QT],
                            in_=ps,
                            func=AF.Identity,
                            bias=bias2[:, m : m + 1],
                        )
                    else:
                        nc.vector.tensor_scalar_add(
                            out=qkvT[:, m, qt * QT : (qt + 1) * QT],
                            in0=ps,
                            scalar1=bias2[:, m : m + 1],
                        )

            # ---- phase C: V token-major via fp8 PE transposes ----
            for nt in range(NK):
                # fp8 transpose writes PSUM in 2-byte granules: the verifier
                # requires output element step 2, so view the tile as
                # [128, 256, 2] and write plane 0. Both cc chunks share the
                # tile so one batched drain covers the full 256 channels.
                ps8 = pro_ps.tile([128, 256, 2], F8, tag="trv", name="ps8")
                for cc in range(CC):
                    nc.tensor.transpose(
                        ps8[:, cc * 128 : (cc + 1) * 128, 0],
                        qkvT[:, 4 + cc, nt * 128 : (nt + 1) * 128],
                        ident8,
                    )
                if nt % 2 == 0:
                    nc.vector.tensor_copy(out=v_tm[:, nt, :], in_=ps8[:, :, 0])
                else:
                    nc.scalar.copy(out=v_tm[:, nt, :], in_=ps8[:, :, 0])

        # ---- phase D: attention + proj + skip, per q tile ----
        with (
            tc.tile_pool(name="lgp", bufs=2, space="PSUM") as lgp,
            tc.tile_pool(name="pmisc", bufs=1, space="PSUM") as pmisc,
            tc.tile_pool(name="avp", bufs=1, space="PSUM") as avp,
            tc.tile_pool(name="expp", bufs=3) as expp,
            tc.tile_pool(name="owork", bufs=2) as owork,
        ):
            def emit_lg2(qt, pair):
                """One [128, 2, 512] logits pair (two DoubleRow matmuls, each
                contracting the full C=256 via the fp8 pair dim) feeding one
                wide ACT exp. Two rotating pair-tiles (4 banks) keep the PE
                ~2 pairs ahead of the exp stream."""
                lg2 = lgp.tile([128, 2, QT], F32, tag="lg2", name="lg2")
                for j in range(2):
                    kt = 2 * pair + j
                    nc.tensor.matmul(
                        lg2[:, j, :],
                        lhsT=qkvT[:, 2:4, kt * 128 : (kt + 1) * 128],
                        rhs=qkvT[:, 0:2, qt * QT : (qt + 1) * QT],
                        start=True,
                        stop=True,
                        perf_mode=DRM,
                    )
                return lg2

            def next_lg2(qt, pair):
                if pair < NP:
                    return emit_lg2(qt, pair)
                if qt + 1 < NQ:
                    return emit_lg2(qt + 1, pair - NP)
                return None

            lg2_cur = emit_lg2(0, 0)
            lg2_nxt = emit_lg2(0, 1)
            for qt in range(NQ):
                av_ps = [
                    avp.tile([128, QT], F32, tag=f"av{cc}", name=f"av{cc}")
                    for cc in range(CC)
                ]
                # denominator bank: matmul writes row 0; the full [128, 512]
                # shape lets the proj reuse this bank at the tail (tag "den")
                den_t = pmisc.tile([128, QT], F32, tag="den", name="den_t")
                den = den_t[0:1, :]

                for pair in range(NP):
                    expT2 = expp.tile([128, 2, QT], F8, tag="expT2",
                                      name="expT2")
                    # one wide exp over both banks; the -4 shift keeps the
                    # fp8 numerator in range and cancels in the softmax.
                    nc.scalar.activation(
                        out=expT2,
                        in_=lg2_cur,
                        func=AF.Exp,
                        scale=1.0 / 16.0,
                        bias=nshift_col,
                    )
                    lg2_cur = lg2_nxt
                    lg2_nxt = next_lg2(qt, pair + 2)
                    for cc in range(CC):
                        nc.tensor.matmul(
                            av_ps[cc],
                            lhsT=v_tm[:, 2 * pair : 2 * pair + 2,
                                      cc * 128 : (cc + 1) * 128],
                            rhs=expT2,
                            start=(pair == 0),
                            stop=(pair == NP - 1),
                            perf_mode=DRM,
                        )
                    nc.tensor.matmul(
                        den,
                        lhsT=ones8,
                        rhs=expT2,
                        start=(pair == 0),
                        stop=(pair == NP - 1),
                        perf_mode=DRM,
                    )

                # ---- tail: softmax denominator -> normalize -> proj ----
                recip_row = owork.tile([1, QT], BF16, tag="recip_row")
                nc.vector.reciprocal(out=recip_row, in_=den)
                rec_ps = pmisc.tile([128, QT], F32, tag="rec", name="rec")
                nc.tensor.matmul(
                    rec_ps, lhsT=ones_col_bf, rhs=recip_row,
                    start=True, stop=True,
                )
                rec_sb = owork.tile([128, QT], BF16, tag="rec_sb")
                nc.vector.tensor_copy(out=rec_sb, in_=rec_ps)
                av_sb = owork.tile([128, CC, QT], BF16, tag="av_sb")
                for cc in range(CC):
                    nc.vector.tensor_mul(
                        out=av_sb[:, cc, :], in0=av_ps[cc], in1=rec_sb
                    )

                # proj token-major into the den/rec banks (both consumed by
                # now): pjA <- rec bank, pjB <- den bank; no output transposes
                pjA = pmisc.tile([128, QT], F32, tag="rec", name="pjA")
                pjB = pmisc.tile([128, QT], F32, tag="den", name="pjB")
                for tt in range(4):
                    bank = pjA if tt < 2 else pjB
                    seg = bank[:, (tt % 2) * C : (tt % 2 + 1) * C]
                    for cc in range(CC):
                        nc.tensor.matmul(
                            seg,
                            lhsT=av_sb[:, cc, tt * 128 : (tt + 1) * 128],
                            rhs=wp_bf[:, cc, :],
                            start=(cc == 0),
                            stop=(cc == CC - 1),
                        )

                out_sb = owork.tile([128, 4, C], F32, tag="out_sb")
                for half in range(2):
                    nc.vector.tensor_add(
                        out=out_sb[:, half * 2 : (half + 1) * 2, :],
                        in0=(pjA if half == 0 else pjB),
                        in1=x_tm[:, qt * 4 + half * 2 : qt * 4 + (half + 1) * 2, :],
                    )
                nc.vector.tensor_add(out=out_sb, in0=out_sb, in1=bp4)
                nc.sync.dma_start(
                    out=out_tok[:, qt * 4 : (qt + 1) * 4, :], in_=out_sb
                )


_NC = None


def _get_nc():
    global _NC
    if _NC is None:
        _NC = _build()
    return _NC


_RUNNER = None
_ZEROS_FN = None

IN_NAMES = ["x", "gn_scale", "gn_bias", "w_qkv", "b_qkv", "w_proj", "b_proj"]


def _get_runner():
    """Cached jitted shard_map executable over the 8 cores (the equivalent of
    run_bass_kernel_spmd's axon path, but built once instead of per call)."""
    global _RUNNER
    if _RUNNER is not None:
        return _RUNNER
    import jax
    from jax.sharding import Mesh, PartitionSpec
    from jax.experimental.shard_map import shard_map
    from concourse import bass2jax

    nc = _get_nc()
    bass2jax.install_neuronx_cc_hook()

    in_names = list(IN_NAMES) + ["out"]
    if nc.partition_id_tensor is not None:
        in_names.append(nc.partition_id_tensor.name)

    def _body_fn(*args):
        operands = list(args)
        if nc.partition_id_tensor is not None:
            operands.append(bass2jax.partition_id_tensor())
        outs = bass2jax._bass_exec_p.bind(
            *operands,
            out_avals=(jax.core.ShapedArray((N, C), np.float32),),
            in_names=tuple(in_names),
            out_names=("out",),
            lowering_input_output_aliases=(),
            sim_require_finite=True,
            sim_require_nnan=True,
            nc=nc,
        )
        return tuple(outs)

    devices = jax.devices()[:N_CORES]
    mesh = Mesh(np.asarray(devices), ("core",))
    in_specs = (PartitionSpec("core"),) * (len(IN_NAMES) + 1)
    out_specs = (PartitionSpec("core"),)
    sharded = jax.jit(
        shard_map(
            _body_fn, mesh=mesh, in_specs=in_specs, out_specs=out_specs,
            check_rep=False,
        ),
        donate_argnums=(len(IN_NAMES),),
        keep_unused=True,
    )
    _RUNNER = sharded
    return _RUNNER


def kernel(x, gn_scale, gn_bias, w_qkv, b_qkv, w_proj, b_proj):
    sharded = _get_runner()
    x = np.ascontiguousarray(np.asarray(x, dtype=np.float32).reshape(B * N, C))
    shared = {
        "gn_scale": np.asarray(gn_scale, np.float32),
        "gn_bias": np.asarray(gn_bias, np.float32),
        "w_qkv": np.ascontiguousarray(np.asarray(w_qkv, np.float32)),
        "b_qkv": np.asarray(b_qkv, np.float32),
        "w_proj": np.ascontiguousarray(np.asarray(w_proj, np.float32)),
        "b_proj": np.asarray(b_proj, np.float32),
    }
    # shard_map slices axis 0 across cores: x gets its own batch; the shared
    # weights are tiled 8x so every core sees an identical copy.
    concat = [x]
    for name in IN_NAMES[1:]:
        a = shared[name]
        concat.append(np.concatenate([a] * N_CORES, axis=0))
    # donated output buffer, created on-device (saves a 32MB host->device
    # transfer through the axon tunnel every call)
    import jax
    import jax.numpy as jnp
    from jax.sharding import Mesh, NamedSharding, PartitionSpec

    global _ZEROS_FN
    if _ZEROS_FN is None:
        mesh = Mesh(np.asarray(jax.devices()[:N_CORES]), ("core",))
        sh = NamedSharding(mesh, PartitionSpec("core"))
        _ZEROS_FN = jax.jit(
            lambda: jnp.zeros((N_CORES * N, C), jnp.float32), out_shardings=sh
        )
    zeros = _ZEROS_FN()
    (out,) = sharded(*concat, zeros)
    return np.asarray(out).reshape(B, H, W, C)
